# revision 8
# baseline (speedup 1.0000x reference)
"""3-layer GAT on 8 trn2 NeuronCores — dst-sharded, zero ReduceScatter.

Nodes split 6272/core; each core OWNS its dst rows, so segment softmax and
aggregation are fully local (no reduce-scatter). Per layer each core projects
its own x rows into a table [6272, elem] holding f (dn cols) + el column,
AllGathers the table (the only collective, 3 total), then processes its own
edges bucketed by (src-half, dst-window): 128-edge chunks are (half, window)
pure so gather indices stay int16 and PSUM accumulation is per-window.

Per chunk: G = dma_gather(table_half, srcl) brings f[src] + el[src];
er[dst] is a LOCAL one-hot matmul (OhT^T @ er_window-column) using the dst
one-hot transposed, built from a DMA partition-broadcast of the dst-local
row; scores exp(leaky_relu(el+er)) are computed group-wide; aggregation and
softmax denominator z accumulate in PSUM per (half, window) and combine in
SBUF. Normalize (+relu / log_softmax) happens right after — no collective.

exp uses no segment-max: scores are O(1) so fp32 exp is safe (softmax is
shift-invariant; matches reference in exact math). h ships fp16, output fp16.
"""

import os
import numpy as np

N, E, DIN, DH, DOUT = 50000, 800000, 256, 128, 64
NCORES = 8
PC = 6272              # nodes per core
NPAD = PC * NCORES     # 50176
WPC = 49               # dst windows per core
P = 128
NHALF = 2
HROWS = NPAD // NHALF  # 25088 table rows per gather half
GMAX = 64              # max chunks per gather group
SUB = 8                # chunks per dstl-row broadcast subtile


def _wrap16(idx, rows=16):
    """dma_gather index layout: idx j at [j%16, j//16] (16 rows, unreplicated)."""
    n = len(idx)
    out = np.zeros((rows, n // 16), dtype=np.int16)
    out[:16, :] = idx.astype(np.int16).reshape(-1, 16).T
    return out


def host_prep(h, src, dst, W1, al1, ar1, W2, al2, ar2, W3, al3, ar3):
    f32 = np.float32
    f16 = np.float16
    h = np.asarray(h, f32)
    src = np.asarray(src, np.int64)
    dst = np.asarray(dst, np.int64)

    hp = np.zeros((NPAD, DIN), f32)
    hp[:N] = h

    own = dst // PC
    half_of = src // HROWS
    w_of = (dst % PC) >> 7

    # shared chunk schedule: nch[h][w] = max over cores, >=1
    cnt = np.zeros((NCORES, NHALF, WPC), np.int64)
    np.add.at(cnt, (own, half_of, w_of), 1)
    nch = np.maximum(1, -(-cnt.max(axis=0) // P))  # [NHALF, WPC]
    TOT = int(nch.sum())

    # chunk -> (half, window), chunk start offsets
    chunk_h = np.zeros(TOT, np.int64)
    chunk_w = np.zeros(TOT, np.int64)
    hw_c0 = np.zeros((NHALF, WPC), np.int64)
    off = 0
    for hh in range(NHALF):
        for w in range(WPC):
            hw_c0[hh, w] = off
            chunk_h[off:off + nch[hh, w]] = hh
            chunk_w[off:off + nch[hh, w]] = w
            off += nch[hh, w]
    assert off == TOT

    # gather groups: runs of <=GMAX chunks within one half; per group the
    # (w, wc0, wc1, is_first_half) runs it contains
    groups = []
    ci = 0
    for hh in range(NHALF):
        h_end = ci + int(nch[hh].sum())
        while ci < h_end:
            c0 = ci
            c1 = min(c0 + GMAX, h_end)
            gwins = []
            j = c0
            while j < c1:
                w = int(chunk_w[j])
                k = j
                while k < c1 and chunk_w[k] == w and chunk_h[k] == hh:
                    k += 1
                first = j == hw_c0[hh, w]
                last = k == hw_c0[hh, w] + nch[hh, w]
                gwins.append((w, j - c0, k - c0, first, last))
                j = k
            groups.append((c0, c1, hh, gwins))
            ci = c1

    # per-core slot arrays
    in_maps = []
    wl1 = (np.asarray(W1, f32) @ np.asarray(al1, f32)).reshape(DIN, 1)
    wr1 = (np.asarray(W1, f32) @ np.asarray(ar1, f32)).reshape(DIN, 1)
    wl2 = (np.asarray(W2, f32) @ np.asarray(al2, f32)).reshape(DH, 1)
    wr2 = (np.asarray(W2, f32) @ np.asarray(ar2, f32)).reshape(DH, 1)
    wl3 = (np.asarray(W3, f32) @ np.asarray(al3, f32)).reshape(DH, 1)
    wr3 = (np.asarray(W3, f32) @ np.asarray(ar3, f32)).reshape(DH, 1)
    iota = np.tile(np.arange(P, dtype=f32), (P, 1))
    iotap = np.arange(P, dtype=f32).reshape(P, 1)
    ident = np.eye(P, dtype=f32)

    for c in range(NCORES):
        sel = np.nonzero(own == c)[0]
        e_src = src[sel]
        e_dst = dst[sel]
        e_h = half_of[sel]
        e_w = w_of[sel]

        # order edges by (half, window); position within run -> slot
        key = e_h * WPC + e_w
        order = np.argsort(key, kind="stable")
        e_src, e_dst, e_h, e_w = (a[order] for a in (e_src, e_dst, e_h, e_w))
        ks = key[order]
        pos = np.arange(len(sel)) - np.searchsorted(ks, ks, side="left")
        slot = hw_c0[e_h, e_w] * P + pos

        srcl = np.zeros(TOT * P, np.int64)
        dstl = np.full(TOT * P, 255, np.int64)   # 255 = pad sentinel
        srcl[slot] = e_src - e_h * HROWS
        dstl[slot] = (e_dst - c * PC) & 127

        m = dict(
            hT=np.ascontiguousarray(hp[c * PC:(c + 1) * PC].T.astype(f16)),
            sidx=_wrap16(srcl),
            dstlc=np.ascontiguousarray(
                dstl.reshape(TOT, P).T.astype(np.uint8)),
            dstlr=dstl.astype(np.uint8).reshape(1, TOT * P),
            W1=np.asarray(W1, f16),
            wl1=wl1.astype(f16), wr1=wr1.astype(f16),
            W2=np.asarray(W2, f32), wl2=wl2, wr2=wr2,
            W3=np.asarray(W3, f32), wl3=wl3, wr3=wr3,
            iota=iota, iotap=iotap, ident=ident,
        )
        in_maps.append(m)

    return dict(in_maps=in_maps, TOT=TOT, groups=groups, nch=nch)


def build_program(prep):
    import concourse.bacc as bacc
    import concourse.mybir as mybir
    import concourse.tile as tile
    from concourse import library_config

    f32 = mybir.dt.float32
    f16 = mybir.dt.float16
    u8 = mybir.dt.uint8
    i16 = mybir.dt.int16
    AF = mybir.ActivationFunctionType
    OP = mybir.AluOpType
    TOT = prep["TOT"]
    groups = prep["groups"]
    maxl = int(os.environ.get("GAT_MAXL", "3"))
    noedge = os.environ.get("GAT_NOEDGE")
    nogather = os.environ.get("GAT_NOGATHER")
    noag = os.environ.get("GAT_NOAG")

    nc = bacc.Bacc("TRN2", target_bir_lowering=False, debug=False,
                   num_devices=NCORES)

    hT_d = nc.dram_tensor("hT", [DIN, PC], f16, kind="ExternalInput")
    sidx_d = nc.dram_tensor("sidx", [16, TOT * 8], i16, kind="ExternalInput")
    dstlc_d = nc.dram_tensor("dstlc", [P, TOT], u8, kind="ExternalInput")
    dstlr_d = nc.dram_tensor("dstlr", [1, TOT * P], u8, kind="ExternalInput")
    W1_d = nc.dram_tensor("W1", [DIN, DH], f16, kind="ExternalInput")
    wl1_d = nc.dram_tensor("wl1", [DIN, 1], f16, kind="ExternalInput")
    wr1_d = nc.dram_tensor("wr1", [DIN, 1], f16, kind="ExternalInput")
    W2_d = nc.dram_tensor("W2", [DH, DH], f32, kind="ExternalInput")
    wl2_d = nc.dram_tensor("wl2", [DH, 1], f32, kind="ExternalInput")
    wr2_d = nc.dram_tensor("wr2", [DH, 1], f32, kind="ExternalInput")
    W3_d = nc.dram_tensor("W3", [DH, DOUT], f32, kind="ExternalInput")
    wl3_d = nc.dram_tensor("wl3", [DH, 1], f32, kind="ExternalInput")
    wr3_d = nc.dram_tensor("wr3", [DH, 1], f32, kind="ExternalInput")
    iota_d = nc.dram_tensor("iota", [P, P], f32, kind="ExternalInput")
    iotap_d = nc.dram_tensor("iotap", [P, 1], f32, kind="ExternalInput")
    ident_d = nc.dram_tensor("ident", [P, P], f32, kind="ExternalInput")
    out_d = nc.dram_tensor("out", [PC, DOUT], f16, kind="ExternalOutput")

    # per-layer: input width, out width, el column, gather elem
    # (uniform elem=192 so all layers share one SBUF gather-buffer tag;
    #  table cols: f 0:dn, ones at dn so z folds into the agg matmul,
    #  el at dn+1)
    LAY = {
        1: dict(din=DIN, dn=DH, elcol=DH + 1, elem=192),
        2: dict(din=DH, dn=DH, elcol=DH + 1, elem=192),
        3: dict(din=DH, dn=DOUT, elcol=DOUT + 1, elem=192),
    }

    with tile.TileContext(nc) as tc:
        with (
            tc.tile_pool(name="sbP", bufs=1) as sbP,
            tc.tile_pool(name="sbG", bufs=2) as sbG,
            tc.tile_pool(name="sbS", bufs=3) as sbS,
            tc.tile_pool(name="psA", bufs=2, space="PSUM") as psA,
            tc.tile_pool(name="psB", bufs=3, space="PSUM") as psB,
            tc.tile_pool(name="psC", bufs=2, space="PSUM") as psC,
            tc.tile_pool(name="dram", bufs=1, space="DRAM") as dram,
        ):
            nc.gpsimd.load_library(library_config.mlp)

            iota = sbP.tile([P, P], f32, tag="iota")
            nc.sync.dma_start(iota[:], iota_d[:])
            iotap = sbP.tile([P, 1], f32, tag="iotap")
            nc.sync.dma_start(iotap[:], iotap_d[:])
            ident = sbP.tile([P, P], f32, tag="ident")
            nc.sync.dma_start(ident[:], ident_d[:])
            ones = sbP.tile([P, 1], f32, tag="ones")
            nc.vector.memset(ones[:], 1.0)

            # weights
            Wt = {1: [sbP.tile([P, DH], f16, tag=f"w1_{k}", name=f"w1_{k}")
                      for k in range(2)]}
            for k in range(2):
                nc.sync.dma_start(Wt[1][k][:], W1_d[k * P:(k + 1) * P, :])
            Wt[2] = [sbP.tile([P, DH], f32, tag="w2", name="w2")]
            nc.sync.dma_start(Wt[2][0][:], W2_d[:])
            Wt[3] = [sbP.tile([P, DOUT], f32, tag="w3", name="w3")]
            nc.sync.dma_start(Wt[3][0][:], W3_d[:])
            wv = {}
            for nm, d, dt_, kt in (("wl1", wl1_d, f16, 2), ("wr1", wr1_d, f16, 2),
                                   ("wl2", wl2_d, f32, 1), ("wr2", wr2_d, f32, 1),
                                   ("wl3", wl3_d, f32, 1), ("wr3", wr3_d, f32, 1)):
                ts = []
                for k in range(kt):
                    t = sbP.tile([P, 1], dt_, tag=f"{nm}_{k}", name=f"{nm}_{k}")
                    nc.sync.dma_start(t[:], d[k * P:(k + 1) * P, :])
                    ts.append(t)
                wv[nm] = ts

            # edge indices: replicate 16 -> 128 partitions on device
            sidx_sb = sbP.tile([P, TOT * 8], i16, tag="sidx")
            for k in range(8):
                nc.sync.dma_start(sidx_sb[16 * k:16 * (k + 1), :], sidx_d[:])
            dstlc_u8 = sbP.tile([P, TOT], u8, tag="dstlc8")
            nc.sync.dma_start(dstlc_u8[:], dstlc_d[:])
            dstlc = sbP.tile([P, TOT], f32, tag="dstlc")
            nc.vector.tensor_copy(dstlc[:], dstlc_u8[:])

            # persistent per-layer state
            xT = sbP.tile([P, PC], f32, tag="xT")
            er_stage = sbP.tile([P, WPC], f32, tag="er_stage")
            agg_sb = sbP.tile([P, WPC * (DH + 1)], f32, tag="agg_sb")

            for rep in range(int(os.environ.get("GAT_REPEAT", "1"))):
              tabs = {l: dram.tile([PC, LAY[l]["elem"]], f32,
                                   name=f"tab{l}_{rep}") for l in (1, 2, 3)}
              fulls = {l: dram.tile([NPAD, LAY[l]["elem"]], f32,
                                    addr_space="Shared", name=f"full{l}_{rep}")
                       for l in (1, 2, 3)}
              for l in (1, 2, 3):
                if l > maxl:
                    break
                L = LAY[l]
                din, dn, elcol, elem = L["din"], L["dn"], L["elcol"], L["elem"]
                KT = din // P

                # ---- projection: table (f + el col) + er_stage ----
                for t in range(WPC):
                    if l == 1:
                        xts = []
                        for k in range(KT):
                            xt = sbS.tile([P, P], f16, tag="hTk")
                            nc.sync.dma_start(
                                xt[:], hT_d[k * P:(k + 1) * P, t * P:(t + 1) * P])
                            xts.append(xt[:])
                    else:
                        xts = [xT[:, t * P:(t + 1) * P]]
                    f_ps = psA.tile([P, dn], f32, space="PSUM", tag="pbig")
                    for k in range(KT):
                        nc.tensor.matmul(f_ps[:], xts[k], Wt[l][k][:],
                                         start=(k == 0), stop=(k == KT - 1))
                    el_ps = psB.tile([P, 1], f32, space="PSUM", tag="pcol")
                    er_ps = psB.tile([P, 1], f32, space="PSUM", tag="pcol")
                    for k in range(KT):
                        nc.tensor.matmul(el_ps[:], xts[k], wv[f"wl{l}"][k][:],
                                         start=(k == 0), stop=(k == KT - 1))
                    for k in range(KT):
                        nc.tensor.matmul(er_ps[:], xts[k], wv[f"wr{l}"][k][:],
                                         start=(k == 0), stop=(k == KT - 1))
                    stage = sbS.tile([P, elem], f32, tag="tstage")
                    nc.vector.memset(stage[:, dn:elem], 0.0)
                    nc.vector.memset(stage[:, dn:dn + 1], 1.0)
                    nc.scalar.copy(stage[:, 0:dn], f_ps[:])
                    nc.vector.tensor_copy(stage[:, elcol:elcol + 1], el_ps[:])
                    nc.vector.tensor_copy(er_stage[:, t:t + 1], er_ps[:])
                    nc.sync.dma_start(tabs[l][t * P:(t + 1) * P, :], stage[:])

                if not noag:
                    nc.gpsimd.collective_compute(
                        "AllGather", mybir.AluOpType.bypass,
                        ins=[tabs[l][:]], outs=[fulls[l][:]],
                        replica_groups=[list(range(NCORES))])

                # ---- edge phase ----
                if noedge:
                    continue
                for (c0, c1, hh, gwins) in groups:
                    nchg = c1 - c0
                    G = sbG.tile([P, GMAX, elem], f32, tag="G")
                    if not nogather:
                        nc.gpsimd.dma_gather(
                            G[:, 0:nchg, :],
                            fulls[l][hh * HROWS:(hh + 1) * HROWS, :],
                            sidx_sb[:, c0 * 8:c1 * 8],
                            nchg * P, nchg * P, elem, single_packet=False)
                    else:
                        nc.vector.memset(G[:], 0.5)

                    # er[dst] per chunk: OhT one-hot (dst on partitions) @ er col
                    erp = psC.tile([P, GMAX], f32, space="PSUM", tag="perw")
                    for s0 in range(0, nchg, SUB):
                        s1 = min(s0 + SUB, nchg)
                        du = sbS.tile([P, SUB * P], u8, tag="drep8")
                        nc.sync.dma_start(
                            du[:, 0:(s1 - s0) * P],
                            dstlr_d[0:1, (c0 + s0) * P:(c0 + s1) * P]
                            .to_broadcast([P, (s1 - s0) * P]))
                        df = sbS.tile([P, SUB * P], f32, tag="drepf")
                        nc.vector.tensor_copy(df[:, 0:(s1 - s0) * P],
                                              du[:, 0:(s1 - s0) * P])
                        oht = sbS.tile([P, SUB * P], f32, tag="oht")
                        nc.vector.tensor_scalar(
                            out=oht[:, 0:(s1 - s0) * P],
                            in0=df[:, 0:(s1 - s0) * P],
                            scalar1=iotap[:], scalar2=None, op0=OP.is_equal)
                        for ci in range(s0, s1):
                            w = next(w_ for (w_, a, b, _f, _l) in gwins
                                     if a <= ci < b)
                            nc.tensor.matmul(
                                erp[:, ci:ci + 1],
                                oht[:, (ci - s0) * P:(ci - s0 + 1) * P],
                                er_stage[:, w:w + 1], start=True, stop=True)

                    # scores: exp(leaky_relu(el[src] + er[dst]))
                    sc = sbS.tile([P, GMAX], f32, tag="sc")
                    nc.vector.tensor_tensor(
                        out=sc[:, 0:nchg],
                        in0=G[:, 0:nchg, elcol:elcol + 1].rearrange(
                            "p c u -> p (c u)"),
                        in1=erp[:, 0:nchg], op=OP.add)
                    lr = sbS.tile([P, GMAX], f32, tag="lr")
                    nc.vector.scalar_tensor_tensor(
                        out=lr[:, 0:nchg], in0=sc[:, 0:nchg], scalar=0.2,
                        in1=sc[:, 0:nchg], op0=OP.mult, op1=OP.max)
                    exw = sbS.tile([P, GMAX], f32, tag="exw")
                    nc.scalar.activation(exw[:, 0:nchg], lr[:, 0:nchg], AF.Exp)

                    # aggregate per (half, window)
                    dz = dn + 1
                    for (w, wc0, wc1, first, last) in gwins:
                        agg = psA.tile([P, dz], f32, space="PSUM", tag="pbig")
                        for ci in range(wc0, wc1):
                            oex = sbS.tile([P, P], f32, tag="oex")
                            nc.vector.tensor_scalar(
                                out=oex[:], in0=iota[:],
                                scalar1=dstlc[:, c0 + ci:c0 + ci + 1],
                                scalar2=exw[:, ci:ci + 1],
                                op0=OP.is_equal, op1=OP.mult)
                            nc.tensor.matmul(agg[:], oex[:], G[:, ci, 0:dz],
                                             start=(ci == wc0), stop=(ci == wc1 - 1))
                        if hh == 0 and first:
                            nc.vector.tensor_copy(
                                agg_sb[:, w * dz:(w + 1) * dz], agg[:])
                        else:
                            nc.vector.tensor_tensor(
                                out=agg_sb[:, w * dz:(w + 1) * dz],
                                in0=agg_sb[:, w * dz:(w + 1) * dz],
                                in1=agg[:], op=OP.add)

                # ---- post: normalize (+relu / log_softmax) ----
                dz = dn + 1
                for w in range(WPC):
                    zc = sbS.tile([P, 1], f32, tag="zc")
                    nc.vector.tensor_scalar(
                        out=zc[:], in0=agg_sb[:, w * dz + dn:w * dz + dn + 1],
                        scalar1=1e-9, scalar2=None, op0=OP.max)
                    zrec = sbS.tile([P, 1], f32, tag="zrec")
                    nc.vector.reciprocal(zrec[:], zc[:])
                    if l < 3:
                        xw = sbS.tile([P, dn], f32, tag="xw")
                        nc.vector.tensor_scalar(
                            out=xw[:], in0=agg_sb[:, w * dz:w * dz + dn],
                            scalar1=zrec[:], scalar2=0.0,
                            op0=OP.mult, op1=OP.max)
                        xtp = psA.tile([P, P], f32, space="PSUM", tag="pbig")
                        nc.tensor.transpose(xtp[:], xw[:], ident[:])
                        nc.scalar.copy(xT[:, w * P:(w + 1) * P], xtp[:])
                    else:
                        xs = sbS.tile([P, DOUT], f32, tag="ls1")
                        nc.vector.tensor_scalar(
                            out=xs[:], in0=agg_sb[:, w * dz:w * dz + dn],
                            scalar1=zrec[:], scalar2=None, op0=OP.mult)
                        mx = sbS.tile([P, 1], f32, tag="mx")
                        nc.vector.tensor_reduce(
                            out=mx[:], in_=xs[:], op=OP.max,
                            axis=mybir.AxisListType.X)
                        xm = sbS.tile([P, DOUT], f32, tag="xm")
                        nc.vector.tensor_scalar(
                            out=xm[:], in0=xs[:], scalar1=mx[:],
                            scalar2=None, op0=OP.subtract)
                        ee = sbS.tile([P, DOUT], f32, tag="ee")
                        nc.scalar.activation(ee[:], xm[:], AF.Exp)
                        se = sbS.tile([P, 1], f32, tag="se")
                        nc.vector.tensor_reduce(
                            out=se[:], in_=ee[:], op=OP.add,
                            axis=mybir.AxisListType.X)
                        ls = sbS.tile([P, 1], f32, tag="lsum")
                        nc.scalar.activation(ls[:], se[:], AF.Ln)
                        fo = sbS.tile([P, DOUT], f16, tag="fout")
                        nc.vector.tensor_scalar(
                            out=fo[:], in0=xm[:], scalar1=ls[:],
                            scalar2=None, op0=OP.subtract)
                        nc.sync.dma_start(out_d[w * P:(w + 1) * P, :], fo[:])

    nc.compile()
    return nc


def kernel(**inputs):
    from concourse.bass_utils import run_bass_kernel_spmd

    prep = host_prep(**inputs)
    nc = build_program(prep)
    res = run_bass_kernel_spmd(nc, prep["in_maps"], core_ids=list(range(NCORES)))
    out = np.concatenate([res.results[c]["out"] for c in range(NCORES)], axis=0)
    return np.ascontiguousarray(out[:N]).astype(np.float32)
